# revision 13
# baseline (speedup 1.0000x reference)
"""Trainium2 Bass kernel for nn_ARTAttrEncoderGCN (3-layer GCN + attr embedder
+ global_add_pool + predictor MLP) on 8 NeuronCores.

Strategy (data parallel over nodes, global SpMM via gather):
  - Nodes are block-partitioned across the 8 cores (12500 each, padded 12544).
  - Per layer: each core computes its slice of hw = h @ W + b (dense, PE),
    AllGathers hw into a full 100352-row gather table in DRAM, then processes
    the edges whose SCATTER target (segment id) lives in its node range:
    dma_gather fetches hw[dst] rows (512B each), a per-chunk selection matrix
    M[e, n] = (srcrel[e] == n) * adj_value[e] built on the vector engine turns
    the segment-sum into PE matmuls accumulating in PSUM (feat-major h tiles).
  - h stays resident in SBUF as hT [128 feat x 12544 nodes] per core.
  - Pooling: PE-transpose h tiles, indicator matmul against per-node graph
    ids, AllGather partial per-graph sums, static assembly, replicated MLP.

All loop structure is static and identical across cores; the data-dependent
chunk layout (computed from the actual inputs at build time) is identical by
construction (per-bucket max over cores, padded with zero-value edges).
"""
import numpy as np
import ml_dtypes

import concourse.bass as bass
import concourse.mybir as mybir
from concourse import bacc
from concourse.masks import make_identity
from concourse.tile import TileContext
from concourse.bass_utils import run_bass_kernel_spmd

P = 128
NC = 8
N_NODES = 100_000
N_EDGES = 1_600_000
F_NODE = 16
F_EDGE = 8
HID = 128
N_GRAPHS = 2048
NPC = N_NODES // NC          # 12500 nodes per core
TILES = (NPC + P - 1) // P   # 98 tiles per core
PN = TILES * P               # 12544 padded nodes per core
TBL = NC * PN                # 100352 gather-table rows
WIN = 32768
NWIN = (TBL + WIN - 1) // WIN   # 4 gather windows
GRP = 4                      # tiles per phase-C group (PSUM concurrency)
NGRP = (TILES + GRP - 1) // GRP
EGRP = 8                     # tiles per embedder group
NEGRP = (TILES + EGRP - 1) // EGRP
G_WIN = 512                  # per-core pooled graph window

LAST_RESULT = None           # BassKernelResults of the last run (for test.py)
_CACHE = {}


def _install_ntff_hook():
    """Best-effort: register the axon NTFF profile hook so BASS_TRACE=1 works."""
    import sys, types
    try:
        import antenv
        if "antenv.axon_hooks" not in sys.modules:
            mod = types.ModuleType("antenv.axon_hooks")
            mod._hook = None
            mod.set_axon_ntff_profile_hook = lambda h: setattr(mod, "_hook", h)
            mod.get_axon_ntff_profile_hook = lambda: mod._hook
            sys.modules["antenv.axon_hooks"] = mod
            antenv.axon_hooks = mod
            from trn_agent_boot.trn_boot import _ntff_profile_via_ctypes
            mod._hook = _ntff_profile_via_ctypes("/opt/axon/libaxon_pjrt.so")
    except Exception:
        pass


def _table_row(g_node):
    c = g_node // NPC
    return c * PN + (g_node - c * NPC)


def _bucketize(tile, win, nwin, *vals):
    """Group edges by (tile, win); returns dict (t, w) -> tuple of val arrays."""
    key = tile * nwin + win
    order = np.argsort(key, kind="stable")
    key_s = key[order]
    bounds = np.searchsorted(key_s, np.arange(TILES * nwin + 1))
    out = {}
    for t in range(TILES):
        for w in range(nwin):
            k = t * nwin + w
            sel = order[bounds[k]:bounds[k + 1]]
            if len(sel):
                out[(t, w)] = tuple(v[sel] for v in vals)
    return out


def _prep(inputs):
    """All host-side preprocessing. Returns (static, in_maps)."""
    inp = {k: np.asarray(v) for k, v in inputs.items()}
    adj_src = inp["adj_index"][0].astype(np.int64)   # scatter target
    adj_dst = inp["adj_index"][1].astype(np.int64)   # gather source
    adj_val = inp["adj_value"].astype(np.float32)
    emb_dst = inp["edge_index"][1].astype(np.int64)  # embedder scatter target
    edge_attr = inp["edge_attr"].astype(np.float32)
    node_attr = inp["node_attr"].astype(np.float32)
    batch = inp["batch"].astype(np.int64)

    # ---- adjacency buckets per core: (tile of src, window of dst-row) ----
    core_adj = []
    cnt = np.zeros((NC, TILES, NWIN), np.int64)
    drow_all = _table_row(adj_dst)
    for c in range(NC):
        m = (adj_src >= c * NPC) & (adj_src < (c + 1) * NPC)
        loc = adj_src[m] - c * NPC
        b = _bucketize(loc // P, drow_all[m] // WIN, NWIN,
                       (loc % P).astype(np.float32),
                       (drow_all[m] % WIN).astype(np.int64),
                       adj_val[m])
        core_adj.append(b)
        for (t, w), v in b.items():
            cnt[c, t, w] = len(v[0])
    CNT = ((cnt + P - 1) // P).max(axis=0)          # [TILES, NWIN] static
    NCH = int(CNT.sum())

    # chunk order: group g -> window w -> tile t in group -> chunks
    # call (g, w) covers all chunks of its tiles for window w
    chunk_tile = []          # tile of each chunk (static)
    calls = []               # per group: list of (w, chunk_start, n_chunks)
    for g in range(NGRP):
        t0, t1 = g * GRP, min((g + 1) * GRP, TILES)
        gcalls = []
        for w in range(NWIN):
            s = len(chunk_tile)
            for t in range(t0, t1):
                chunk_tile.extend([t] * int(CNT[t, w]))
            if len(chunk_tile) > s:
                gcalls.append((w, s, len(chunk_tile) - s))
        calls.append(gcalls)
    assert len(chunk_tile) == NCH
    # per-tile first/last chunk position (for PSUM start/stop flags)
    first_chunk = {}
    last_chunk = {}
    for q, t in enumerate(chunk_tile):
        first_chunk.setdefault(t, q)
        last_chunk[t] = q

    # ---- embedder buckets: tile of dst only ----
    core_emb = []
    ecnt = np.zeros((NC, TILES), np.int64)
    for c in range(NC):
        m = (emb_dst >= c * NPC) & (emb_dst < (c + 1) * NPC)
        loc = emb_dst[m] - c * NPC
        eidx = np.nonzero(m)[0]
        b = _bucketize(loc // P, np.zeros(len(loc), np.int64), 1,
                       (loc % P).astype(np.float32), eidx)
        core_emb.append(b)
        for (t, _w), v in b.items():
            ecnt[c, t] = len(v[0])
    ECNT = ((ecnt + P - 1) // P).max(axis=0)        # [TILES]
    NECH = int(ECNT.sum())
    e_off = np.zeros(TILES + 1, np.int64)
    e_off[1:] = np.cumsum(ECNT)

    # ---- per-core graph windows ----
    g_lo = [int(batch[c * NPC]) for c in range(NC)]
    for c in range(NC):
        span = int(batch[(c + 1) * NPC - 1]) - g_lo[c] + 1
        assert span <= G_WIN, f"graph span {span} exceeds {G_WIN}"

    # ---- pack per-core arrays ----
    in_maps = []
    weights = dict(
        W_node=inp["W_node"].astype(np.float32),
        W_edge=inp["W_edge"].astype(np.float32),
        W1b=inp["W1"].astype(ml_dtypes.bfloat16),
        W2b=inp["W2"].astype(ml_dtypes.bfloat16),
        W3b=inp["W3"].astype(ml_dtypes.bfloat16),
        b_embed=inp["b_embed"].astype(np.float32).reshape(1, HID),
        W1=inp["W1"].astype(np.float32), b1=inp["b1"].astype(np.float32).reshape(1, HID),
        W2=inp["W2"].astype(np.float32), b2=inp["b2"].astype(np.float32).reshape(1, HID),
        W3=inp["W3"].astype(np.float32), b3=inp["b3"].astype(np.float32).reshape(1, HID),
        Wp1=inp["Wp1"].astype(np.float32), bp1=inp["bp1"].astype(np.float32).reshape(1, HID),
        Wp2=inp["Wp2"].astype(np.float32), bp2=inp["bp2"].astype(np.float32).reshape(1, 1),
    )
    BF = ml_dtypes.bfloat16
    for c in range(NC):
        srcrel = np.zeros((P, NCH), np.int64)
        adjv = np.zeros((P, NCH), np.float32)
        idx_flat = np.zeros(NCH * P, np.int64)
        q = 0
        for g in range(NGRP):
            t0, t1 = g * GRP, min((g + 1) * GRP, TILES)
            for w in range(NWIN):
                for t in range(t0, t1):
                    nchk = int(CNT[t, w])
                    if nchk == 0:
                        continue
                    if (t, w) in core_adj[c]:
                        s, r, v = core_adj[c][(t, w)]
                    else:
                        s = np.zeros(0, np.float32); r = np.zeros(0, np.int64); v = np.zeros(0, np.float32)
                    ntot = nchk * P
                    s2 = np.zeros(ntot, np.float32); r2 = np.zeros(ntot, np.int64); v2 = np.zeros(ntot, np.float32)
                    s2[:len(s)] = s; r2[:len(r)] = r; v2[:len(v)] = v
                    srcrel[:, q:q + nchk] = s2.reshape(nchk, P).T.astype(np.int64)
                    adjv[:, q:q + nchk] = v2.reshape(nchk, P).T
                    idx_flat[q * P:(q + nchk) * P] = r2
                    q += nchk
        assert q == NCH
        assert idx_flat.max() < WIN and idx_flat.min() >= 0
        gi = idx_flat.astype(np.int16).reshape(NCH * P // 16, 16).T  # [16, NCH*8]
        gidx = np.tile(gi, (8, 1)).copy()

        Mg = np.zeros((P, NCH, P), ml_dtypes.bfloat16)
        pp_, qq_ = np.meshgrid(np.arange(P), np.arange(NCH), indexing="ij")
        Mg[pp_, qq_, srcrel] = adjv.astype(ml_dtypes.bfloat16)
        dloc = np.zeros((P, NECH), np.int64)
        ea = np.zeros((P, NECH, F_EDGE), np.float32)
        for t in range(TILES):
            nchk = int(ECNT[t])
            if nchk == 0:
                continue
            if (t, 0) in core_emb[c]:
                dl, ei = core_emb[c][(t, 0)]
            else:
                dl = np.zeros(0, np.float32); ei = np.zeros(0, np.int64)
            ntot = nchk * P
            d2 = np.zeros(ntot, np.int64)
            a2 = np.zeros((ntot, F_EDGE), np.float32)
            d2[:len(dl)] = dl.astype(np.int64)
            a2[:len(ei)] = edge_attr[ei]
            s0 = int(e_off[t])
            dloc[:, s0:s0 + nchk] = d2.reshape(nchk, P).T
            ea[:, s0:s0 + nchk, :] = a2.reshape(nchk, P, F_EDGE).transpose(1, 0, 2)

        naT = np.zeros((F_NODE, PN), np.float32)
        naT[:, :NPC] = node_attr[c * NPC:(c + 1) * NPC].T
        gloc = np.zeros((P, TILES), np.float32)
        gl = (batch[c * NPC:(c + 1) * NPC] - g_lo[c]).astype(np.float32)
        gl_pad = np.zeros(PN, np.float32)
        gl_pad[:NPC] = gl
        gloc[:, :] = gl_pad.reshape(TILES, P).T

        pe_, qe_ = np.meshgrid(np.arange(P), np.arange(NECH), indexing="ij")
        Me = np.zeros((P, NECH, P), ml_dtypes.bfloat16)
        Me[pe_, qe_, dloc] = np.asarray(1.0, ml_dtypes.bfloat16)
        im = dict(naT=naT, ea=ea.astype(BF), Mg=Mg, Me=Me, gidx=gidx, gloc=gloc)
        im.update(weights)
        in_maps.append(im)

    static = dict(NCH=NCH, NECH=NECH, CNT=CNT, ECNT=ECNT, e_off=e_off,
                  calls=calls, chunk_tile=chunk_tile,
                  first_chunk=first_chunk, last_chunk=last_chunk, g_lo=g_lo)
    return static, in_maps


def _build(st):
    NCH, NECH = st["NCH"], st["NECH"]
    CNT, ECNT, e_off = st["CNT"], st["ECNT"], st["e_off"]
    calls, chunk_tile = st["calls"], st["chunk_tile"]
    first_chunk, last_chunk = st["first_chunk"], st["last_chunk"]
    g_lo = st["g_lo"]
    f32 = mybir.dt.float32
    bf16 = mybir.dt.bfloat16

    nc = bacc.Bacc(None, num_devices=NC, debug=False, num_swdge_queues=4, dynamic_dma_scratch_size=65536)
    # ---- inputs ----
    naT_d = nc.dram_tensor("naT", [F_NODE, PN], f32, kind="ExternalInput")
    ea_d = nc.dram_tensor("ea", [P, NECH, F_EDGE], bf16, kind="ExternalInput")
    Mg_d = nc.dram_tensor("Mg", [P, NCH, P], bf16, kind="ExternalInput")
    Me_d = nc.dram_tensor("Me", [P, NECH, P], bf16, kind="ExternalInput")
    gidx_d = nc.dram_tensor("gidx", [P, NCH * 8], mybir.dt.int16, kind="ExternalInput")
    gloc_d = nc.dram_tensor("gloc", [P, TILES], f32, kind="ExternalInput")
    Wn_d = nc.dram_tensor("W_node", [F_NODE, HID], f32, kind="ExternalInput")
    We_d = nc.dram_tensor("W_edge", [F_EDGE, HID], f32, kind="ExternalInput")
    bemb_d = nc.dram_tensor("b_embed", [1, HID], f32, kind="ExternalInput")
    Wl_d = [nc.dram_tensor(f"W{i}", [HID, HID], f32, kind="ExternalInput") for i in (1, 2, 3)]
    Wlb_d = [nc.dram_tensor(f"W{i}b", [HID, HID], bf16, kind="ExternalInput") for i in (1, 2, 3)]
    bl_d = [nc.dram_tensor(f"b{i}", [1, HID], f32, kind="ExternalInput") for i in (1, 2, 3)]
    Wp1_d = nc.dram_tensor("Wp1", [HID, HID], f32, kind="ExternalInput")
    bp1_d = nc.dram_tensor("bp1", [1, HID], f32, kind="ExternalInput")
    Wp2_d = nc.dram_tensor("Wp2", [HID, 1], f32, kind="ExternalInput")
    bp2_d = nc.dram_tensor("bp2", [1, 1], f32, kind="ExternalInput")
    out_d = nc.dram_tensor("out", [N_GRAPHS, 1], f32, kind="ExternalOutput")

    MAXCALL = max(nn for gc in calls for (_w, _s, nn) in gc)

    with TileContext(nc) as tc:
        with tc.tile_pool(name="persist", bufs=1) as pp, \
             tc.tile_pool(name="dram", bufs=1, space="DRAM") as dr:
            hT = pp.tile([P, PN], bf16)                     # resident h (feat-major)
            gloc_sb = pp.tile([P, TILES], f32)
            iota_sb = pp.tile([P, P], bf16)
            iota512_sb = pp.tile([P, G_WIN], f32)
            ones_sb = pp.tile([1, G_WIN], f32)
            ident_sb = pp.tile([P, P], bf16)
            Wn_sb = pp.tile([F_NODE, HID], f32)
            We_sb = pp.tile([F_EDGE, HID], f32)
            bemb_sb = pp.tile([1, HID], f32)
            Wl_sb = [pp.tile([HID, HID], bf16, name=f"Wl{i}") for i in range(3)]
            bl_sb = [pp.tile([1, HID], f32, name=f"bl{i}") for i in range(3)]
            Wp1_sb = pp.tile([HID, HID], f32)
            bp1_sb = pp.tile([1, HID], f32)
            Wp2_sb = pp.tile([HID, 1], f32)
            bp2_sb = pp.tile([1, 1], f32)

            hwslice = dr.tile([PN, HID], bf16)
            tables = [dr.tile([TBL, HID], bf16, addr_space="Shared", name=f"table{l}") for l in range(3)]
            fpslice = dr.tile([P, G_WIN], f32)
            fp_all = dr.tile([NC * P, G_WIN], f32, addr_space="Shared")

            nc.sync.dma_start(gloc_sb[:], gloc_d[:])
            nc.sync.dma_start(Wn_sb[:], Wn_d[:])
            nc.sync.dma_start(We_sb[:], We_d[:])
            nc.sync.dma_start(bemb_sb[:], bemb_d[:])
            for i in range(3):
                nc.sync.dma_start(Wl_sb[i][:], Wlb_d[i][:])
                nc.sync.dma_start(bl_sb[i][:], bl_d[i][:])
            nc.sync.dma_start(Wp1_sb[:], Wp1_d[:])
            nc.sync.dma_start(bp1_sb[:], bp1_d[:])
            nc.sync.dma_start(Wp2_sb[:], Wp2_d[:])
            nc.sync.dma_start(bp2_sb[:], bp2_d[:])
            nc.gpsimd.iota(iota_sb[:], pattern=[[1, P]], base=0,
                           channel_multiplier=0, allow_small_or_imprecise_dtypes=True)
            nc.gpsimd.iota(iota512_sb[:], pattern=[[1, G_WIN]], base=0,
                           channel_multiplier=0, allow_small_or_imprecise_dtypes=True)
            nc.gpsimd.memset(ones_sb[:], 1.0)
            make_identity(nc, ident_sb[:])

            # Shared pools for phase A (hw = h @ W + b), open across all stages
            with tc.tile_pool(name="g_hw", bufs=3) as ghw, \
                 tc.tile_pool(name="g_call", bufs=5) as gcp, \
                 tc.tile_pool(name="g_m", bufs=4) as gmp, \
                 tc.tile_pool(name="p_sb", bufs=4) as psb, \
                 tc.tile_pool(name="g_psa", bufs=1, space="PSUM") as psa:

                def phase_a(layer, t):
                    # hw(layer) for tile t -> hwslice rows (bf16), node-major
                    apsum = psa.tile([P, HID], f32, tag="hw", name=f"hw{layer}_{t}")
                    nc.tensor.matmul(apsum[:], hT[:, t * P:(t + 1) * P],
                                     Wl_sb[layer][:], start=True, stop=False)
                    nc.tensor.matmul(apsum[:], ones_sb[:, :P], bl_sb[layer][:],
                                     start=False, stop=True)
                    hw_sb = ghw.tile([P, HID], bf16, tag="hwsb", name=f"hwsb{layer}_{t}")
                    nc.vector.tensor_copy(hw_sb[:], apsum[:])
                    nc.sync.dma_start(hwslice[t * P:(t + 1) * P, :], hw_sb[:])

                def emit_ag(layer):
                    nc.gpsimd.collective_compute(
                        "AllGather", mybir.AluOpType.bypass,
                        replica_groups=[list(range(NC))],
                        ins=[hwslice[:].opt()], outs=[tables[layer][:].opt()])

                # ================= Stage 1: attr embedder (+ phase A of layer 0) ====
                with tc.tile_pool(name="e_sb", bufs=2) as esb, \
                     tc.tile_pool(name="e_na", bufs=3) as ena, \
                     tc.tile_pool(name="e_m", bufs=2) as emp, \
                     tc.tile_pool(name="e_q", bufs=3) as eqp, \
                     tc.tile_pool(name="e_psq", bufs=2, space="PSUM") as psq, \
                     tc.tile_pool(name="e_pso", bufs=4, space="PSUM") as pso:
                    for g in range(NEGRP):
                        t0, t1 = g * EGRP, min((g + 1) * EGRP, TILES)
                        c0, c1 = int(e_off[t0]), int(e_off[t1])
                        if c1 > c0:
                            ea_sb = esb.tile([P, c1 - c0, F_EDGE], bf16, tag="ea")
                            nc.sync.dma_start(ea_sb[:], ea_d[:, c0:c1, :])
                        for t in range(t0, t1):
                            nchk = int(ECNT[t])
                            qpsum = psq.tile([F_EDGE, P], f32, tag="q")
                            if nchk:
                                e0 = int(e_off[t])
                                m01 = emp.tile([P, nchk, P], bf16, tag="m01")
                                nc.sync.dma_start(m01[:], Me_d[:, e0:e0 + nchk, :])
                            for j in range(nchk):
                                q = int(e_off[t]) + j
                                nc.tensor.matmul(
                                    qpsum[:], ea_sb[:, q - c0, :], m01[:, j, :],
                                    start=(j == 0), stop=(j == nchk - 1))
                            q_sb = eqp.tile([F_EDGE, P], f32, tag="qsb")
                            if nchk:
                                nc.vector.tensor_copy(q_sb[:], qpsum[:])
                            else:
                                nc.gpsimd.memset(q_sb[:], 0.0)
                            na_sb = ena.tile([F_NODE, P], f32, tag="na")
                            nc.sync.dma_start(na_sb[:], naT_d[:, t * P:(t + 1) * P])
                            opsum = pso.tile([P, P], f32, tag="eo")
                            nc.tensor.matmul(opsum[:], We_sb[:], q_sb[:], start=True, stop=False)
                            nc.tensor.matmul(opsum[:], Wn_sb[:], na_sb[:], start=False, stop=False)
                            nc.tensor.matmul(opsum[:], bemb_sb[:], ones_sb[:, :P], start=False, stop=True)
                            nc.scalar.activation(hT[:, t * P:(t + 1) * P], opsum[:],
                                                 mybir.ActivationFunctionType.Relu)
                            phase_a(0, t)
                emit_ag(0)

                # ================= Stage 2: GCN layers (phase A/pooling interleaved) =
                with tc.tile_pool(name="g_psc", bufs=4, space="PSUM") as psc, \
                     tc.tile_pool(name="p_ps", bufs=2, space="PSUM") as pst, \
                     tc.tile_pool(name="p_fp", bufs=1, space="PSUM") as pfp:
                    fp_psum = pfp.tile([P, G_WIN], f32)

                    def pool_tile(t):
                        tpsum = pst.tile([P, P], bf16, tag="tp", name=f"tp{t}")
                        nc.tensor.transpose(out=tpsum[:], in_=hT[:, t * P:(t + 1) * P],
                                            identity=ident_sb[:])
                        hnm = psb.tile([P, P], f32, tag="hnm", bufs=3, name=f"hnm{t}")
                        nc.vector.tensor_copy(hnm[:], tpsum[:])
                        b01 = psb.tile([P, G_WIN], f32, tag="b01", bufs=3, name=f"b01{t}")
                        nc.vector.tensor_scalar(
                            out=b01[:], in0=iota512_sb[:],
                            scalar1=gloc_sb[:, t:t + 1], scalar2=None,
                            op0=mybir.AluOpType.is_equal)
                        nc.tensor.matmul(fp_psum[:], hnm[:], b01[:],
                                         start=(t == 0), stop=(t == TILES - 1),
                                         skip_group_check=True)

                    for layer in range(3):
                        for g in range(NGRP):
                            t0, t1 = g * GRP, min((g + 1) * GRP, TILES)
                            g0 = calls[g][0][1]
                            gn = calls[g][-1][1] + calls[g][-1][2] - g0
                            gix = gmp.tile([P, gn * 8], mybir.dt.int16, tag="gix")
                            nc.sync.dma_start(gix[:], gidx_d[:, g0 * 8:(g0 + gn) * 8])
                            cpsum = {t: psc.tile([P, P], f32, tag="cp", name=f"cp{layer}_{t}")
                                     for t in range(t0, t1)}
                            for (w, s, nn) in calls[g]:
                                gbuf = gcp.tile([P, MAXCALL, P], bf16, tag="gc")
                                base = w * WIN
                                rows = min(WIN, TBL - base)
                                nc.gpsimd.dma_gather(
                                    out_ap=gbuf[:, :nn, :],
                                    in_ap=tables[layer][base:base + rows, :],
                                    idxs_ap=gix[:, (s - g0) * 8:(s - g0 + nn) * 8],
                                    num_idxs=nn * P, num_idxs_reg=nn * P,
                                    elem_size=HID, single_packet=False, queue_num=w)
                                mt = gmp.tile([P, MAXCALL, P], bf16, tag="m")
                                nc.sync.dma_start(mt[:, :nn, :], Mg_d[:, s:s + nn, :])
                                for j in range(nn):
                                    q = s + j
                                    t = chunk_tile[q]
                                    nc.tensor.matmul(
                                        cpsum[t][:], gbuf[:, j, :], mt[:, j, :],
                                        start=(q == first_chunk[t]),
                                        stop=(q == last_chunk[t]))
                            for t in range(t0, t1):
                                nc.scalar.activation(hT[:, t * P:(t + 1) * P],
                                                     cpsum[t][:],
                                                     mybir.ActivationFunctionType.Relu)
                                if layer < 2:
                                    phase_a(layer + 1, t)
                                else:
                                    pool_tile(t)
                        if layer < 2:
                            emit_ag(layer + 1)

                # ================= Stage 3: pooled-sum exchange + MLP ================
                with tc.tile_pool(name="p_mlp", bufs=2, space="PSUM") as pml:
                    fp_sb = psb.tile([P, G_WIN], f32, tag="fpsb", bufs=1)
                    nc.vector.tensor_copy(fp_sb[:], fp_psum[:])
                    nc.sync.dma_start(fpslice[:], fp_sb[:])
                    nc.gpsimd.collective_compute(
                        "AllGather", mybir.AluOpType.bypass,
                        replica_groups=[list(range(NC))],
                        ins=[fpslice[:].opt()], outs=[fp_all[:].opt()])
                    # static assembly of full fpT [128, N_GRAPHS]
                    fpT = psb.tile([P, N_GRAPHS], f32, tag="fpT", bufs=1)
                    nc.vector.memset(fpT[:], 0.0)
                    for c in range(NC):
                        wdt = min(G_WIN, N_GRAPHS - g_lo[c])
                        part = psb.tile([P, G_WIN], f32, tag="fpp", bufs=2)
                        nc.sync.dma_start(part[:], fp_all[c * P:(c + 1) * P, :])
                        nc.vector.tensor_add(fpT[:, g_lo[c]:g_lo[c] + wdt],
                                             fpT[:, g_lo[c]:g_lo[c] + wdt],
                                             part[:, :wdt])
                    # z = relu(fp @ Wp1 + bp1); out = z @ Wp2 + bp2
                    zT = psb.tile([P, N_GRAPHS], f32, tag="zT", bufs=1)
                    o_sb = psb.tile([1, N_GRAPHS], f32, tag="osb", bufs=1)
                    for j in range(N_GRAPHS // G_WIN):
                        sl = slice(j * G_WIN, (j + 1) * G_WIN)
                        zp = pml.tile([P, G_WIN], f32, tag="zp")
                        nc.tensor.matmul(zp[:], Wp1_sb[:], fpT[:, sl], start=True, stop=False)
                        nc.tensor.matmul(zp[:], bp1_sb[:], ones_sb[:], start=False, stop=True)
                        nc.scalar.activation(zT[:, sl], zp[:], mybir.ActivationFunctionType.Relu)
                        op = pml.tile([1, G_WIN], f32, tag="op")
                        nc.tensor.matmul(op[:], Wp2_sb[:], zT[:, sl], start=True, stop=False)
                        nc.tensor.matmul(op[:], bp2_sb[:], ones_sb[:], start=False, stop=True)
                        nc.vector.tensor_copy(o_sb[:, sl], op[:])
                    nc.sync.dma_start(out_d[:, 0][None, :], o_sb[:])
    nc.finalize()
    return nc


def kernel(**inputs):
    global LAST_RESULT
    _install_ntff_hook()
    st, in_maps = _prep(inputs)
    key = (st["NCH"], st["NECH"])
    if key not in _CACHE:
        _CACHE[key] = _build(st)
    nc = _CACHE[key]
    import os
    trace = bool(os.environ.get("BASS_TRACE"))
    r = run_bass_kernel_spmd(nc, in_maps, core_ids=list(range(NC)), trace=trace)
    LAST_RESULT = r
    return r.results[0]["out"]


# revision 14
# speedup vs baseline: 1.0225x; 1.0225x over previous
"""Trainium2 Bass kernel for nn_ARTAttrEncoderGCN (3-layer GCN + attr embedder
+ global_add_pool + predictor MLP) on 8 NeuronCores.

Strategy (data parallel over nodes, global SpMM via gather):
  - Nodes are block-partitioned across the 8 cores (12500 each, padded 12544).
  - Per layer: each core computes its slice of hw = h @ W + b (dense, PE),
    AllGathers hw into a full 100352-row gather table in DRAM, then processes
    the edges whose SCATTER target (segment id) lives in its node range:
    dma_gather fetches hw[dst] rows (512B each), a per-chunk selection matrix
    M[e, n] = (srcrel[e] == n) * adj_value[e] built on the vector engine turns
    the segment-sum into PE matmuls accumulating in PSUM (feat-major h tiles).
  - h stays resident in SBUF as hT [128 feat x 12544 nodes] per core.
  - Pooling: PE-transpose h tiles, indicator matmul against per-node graph
    ids, AllGather partial per-graph sums, static assembly, replicated MLP.

All loop structure is static and identical across cores; the data-dependent
chunk layout (computed from the actual inputs at build time) is identical by
construction (per-bucket max over cores, padded with zero-value edges).
"""
import numpy as np
import ml_dtypes

import concourse.bass as bass
import concourse.mybir as mybir
from concourse import bacc
from concourse.masks import make_identity
from concourse.tile import TileContext
from concourse.bass_utils import run_bass_kernel_spmd

P = 128
NC = 8
N_NODES = 100_000
N_EDGES = 1_600_000
F_NODE = 16
F_EDGE = 8
HID = 128
N_GRAPHS = 2048
NPC = N_NODES // NC          # 12500 nodes per core
TILES = (NPC + P - 1) // P   # 98 tiles per core
PN = TILES * P               # 12544 padded nodes per core
TBL = NC * PN                # 100352 gather-table rows
WIN = 32768
NWIN = (TBL + WIN - 1) // WIN   # 4 gather windows
GRP = 4                      # tiles per phase-C group (PSUM concurrency)
NGRP = (TILES + GRP - 1) // GRP
EGRP = 8                     # tiles per embedder group
NEGRP = (TILES + EGRP - 1) // EGRP
G_WIN = 512                  # per-core pooled graph window

LAST_RESULT = None           # BassKernelResults of the last run (for test.py)
_CACHE = {}


def _install_ntff_hook():
    """Best-effort: register the axon NTFF profile hook so BASS_TRACE=1 works."""
    import sys, types
    try:
        import antenv
        if "antenv.axon_hooks" not in sys.modules:
            mod = types.ModuleType("antenv.axon_hooks")
            mod._hook = None
            mod.set_axon_ntff_profile_hook = lambda h: setattr(mod, "_hook", h)
            mod.get_axon_ntff_profile_hook = lambda: mod._hook
            sys.modules["antenv.axon_hooks"] = mod
            antenv.axon_hooks = mod
            from trn_agent_boot.trn_boot import _ntff_profile_via_ctypes
            mod._hook = _ntff_profile_via_ctypes("/opt/axon/libaxon_pjrt.so")
    except Exception:
        pass


def _table_row(g_node):
    c = g_node // NPC
    return c * PN + (g_node - c * NPC)


def _bucketize(tile, win, nwin, *vals):
    """Group edges by (tile, win); returns dict (t, w) -> tuple of val arrays."""
    key = tile * nwin + win
    order = np.argsort(key, kind="stable")
    key_s = key[order]
    bounds = np.searchsorted(key_s, np.arange(TILES * nwin + 1))
    out = {}
    for t in range(TILES):
        for w in range(nwin):
            k = t * nwin + w
            sel = order[bounds[k]:bounds[k + 1]]
            if len(sel):
                out[(t, w)] = tuple(v[sel] for v in vals)
    return out


def _prep(inputs):
    """All host-side preprocessing. Returns (static, in_maps)."""
    inp = {k: np.asarray(v) for k, v in inputs.items()}
    adj_src = inp["adj_index"][0].astype(np.int64)   # scatter target
    adj_dst = inp["adj_index"][1].astype(np.int64)   # gather source
    adj_val = inp["adj_value"].astype(np.float32)
    emb_dst = inp["edge_index"][1].astype(np.int64)  # embedder scatter target
    edge_attr = inp["edge_attr"].astype(np.float32)
    node_attr = inp["node_attr"].astype(np.float32)
    batch = inp["batch"].astype(np.int64)

    # ---- adjacency buckets per core: (tile of src, window of dst-row) ----
    core_adj = []
    cnt = np.zeros((NC, TILES, NWIN), np.int64)
    drow_all = _table_row(adj_dst)
    for c in range(NC):
        m = (adj_src >= c * NPC) & (adj_src < (c + 1) * NPC)
        loc = adj_src[m] - c * NPC
        b = _bucketize(loc // P, drow_all[m] // WIN, NWIN,
                       (loc % P).astype(np.float32),
                       (drow_all[m] % WIN).astype(np.int64),
                       adj_val[m])
        core_adj.append(b)
        for (t, w), v in b.items():
            cnt[c, t, w] = len(v[0])
    CNT = ((cnt + P - 1) // P).max(axis=0)          # [TILES, NWIN] static
    NCH = int(CNT.sum())

    # chunk order: group g -> window w -> tile t in group -> chunks
    # call (g, w) covers all chunks of its tiles for window w
    chunk_tile = []          # tile of each chunk (static)
    calls = []               # per group: list of (w, chunk_start, n_chunks)
    for g in range(NGRP):
        t0, t1 = g * GRP, min((g + 1) * GRP, TILES)
        gcalls = []
        for w in range(NWIN):
            s = len(chunk_tile)
            for t in range(t0, t1):
                chunk_tile.extend([t] * int(CNT[t, w]))
            if len(chunk_tile) > s:
                gcalls.append((w, s, len(chunk_tile) - s))
        calls.append(gcalls)
    assert len(chunk_tile) == NCH
    # per-tile first/last chunk position (for PSUM start/stop flags)
    first_chunk = {}
    last_chunk = {}
    for q, t in enumerate(chunk_tile):
        first_chunk.setdefault(t, q)
        last_chunk[t] = q

    # ---- embedder buckets: tile of dst only ----
    core_emb = []
    ecnt = np.zeros((NC, TILES), np.int64)
    for c in range(NC):
        m = (emb_dst >= c * NPC) & (emb_dst < (c + 1) * NPC)
        loc = emb_dst[m] - c * NPC
        eidx = np.nonzero(m)[0]
        b = _bucketize(loc // P, np.zeros(len(loc), np.int64), 1,
                       (loc % P).astype(np.float32), eidx)
        core_emb.append(b)
        for (t, _w), v in b.items():
            ecnt[c, t] = len(v[0])
    ECNT = ((ecnt + P - 1) // P).max(axis=0)        # [TILES]
    NECH = int(ECNT.sum())
    e_off = np.zeros(TILES + 1, np.int64)
    e_off[1:] = np.cumsum(ECNT)

    # ---- per-core graph windows ----
    g_lo = [int(batch[c * NPC]) for c in range(NC)]
    for c in range(NC):
        span = int(batch[(c + 1) * NPC - 1]) - g_lo[c] + 1
        assert span <= G_WIN, f"graph span {span} exceeds {G_WIN}"

    # ---- pack per-core arrays ----
    in_maps = []
    weights = dict(
        W_node=inp["W_node"].astype(np.float32),
        W_edge=inp["W_edge"].astype(np.float32),
        W1b=inp["W1"].astype(ml_dtypes.bfloat16),
        W2b=inp["W2"].astype(ml_dtypes.bfloat16),
        W3b=inp["W3"].astype(ml_dtypes.bfloat16),
        b_embed=inp["b_embed"].astype(np.float32).reshape(1, HID),
        W1=inp["W1"].astype(np.float32), b1=inp["b1"].astype(np.float32).reshape(1, HID),
        W2=inp["W2"].astype(np.float32), b2=inp["b2"].astype(np.float32).reshape(1, HID),
        W3=inp["W3"].astype(np.float32), b3=inp["b3"].astype(np.float32).reshape(1, HID),
        Wp1=inp["Wp1"].astype(np.float32), bp1=inp["bp1"].astype(np.float32).reshape(1, HID),
        Wp2=inp["Wp2"].astype(np.float32), bp2=inp["bp2"].astype(np.float32).reshape(1, 1),
    )
    BF = ml_dtypes.bfloat16
    for c in range(NC):
        srcrel = np.zeros((P, NCH), np.int64)
        adjv = np.zeros((P, NCH), np.float32)
        idx_flat = np.zeros(NCH * P, np.int64)
        q = 0
        for g in range(NGRP):
            t0, t1 = g * GRP, min((g + 1) * GRP, TILES)
            for w in range(NWIN):
                for t in range(t0, t1):
                    nchk = int(CNT[t, w])
                    if nchk == 0:
                        continue
                    if (t, w) in core_adj[c]:
                        s, r, v = core_adj[c][(t, w)]
                    else:
                        s = np.zeros(0, np.float32); r = np.zeros(0, np.int64); v = np.zeros(0, np.float32)
                    ntot = nchk * P
                    s2 = np.zeros(ntot, np.float32); r2 = np.zeros(ntot, np.int64); v2 = np.zeros(ntot, np.float32)
                    s2[:len(s)] = s; r2[:len(r)] = r; v2[:len(v)] = v
                    srcrel[:, q:q + nchk] = s2.reshape(nchk, P).T.astype(np.int64)
                    adjv[:, q:q + nchk] = v2.reshape(nchk, P).T
                    idx_flat[q * P:(q + nchk) * P] = r2
                    q += nchk
        assert q == NCH
        assert idx_flat.max() < WIN and idx_flat.min() >= 0
        gi = idx_flat.astype(np.int16).reshape(NCH * P // 16, 16).T  # [16, NCH*8]
        gidx = np.tile(gi, (8, 1)).copy()

        Mg = np.zeros((P, NCH, P), ml_dtypes.bfloat16)
        pp_, qq_ = np.meshgrid(np.arange(P), np.arange(NCH), indexing="ij")
        Mg[pp_, qq_, srcrel] = adjv.astype(ml_dtypes.bfloat16)
        dloc = np.zeros((P, NECH), np.int64)
        ea = np.zeros((P, NECH, F_EDGE), np.float32)
        for t in range(TILES):
            nchk = int(ECNT[t])
            if nchk == 0:
                continue
            if (t, 0) in core_emb[c]:
                dl, ei = core_emb[c][(t, 0)]
            else:
                dl = np.zeros(0, np.float32); ei = np.zeros(0, np.int64)
            ntot = nchk * P
            d2 = np.zeros(ntot, np.int64)
            a2 = np.zeros((ntot, F_EDGE), np.float32)
            d2[:len(dl)] = dl.astype(np.int64)
            a2[:len(ei)] = edge_attr[ei]
            s0 = int(e_off[t])
            dloc[:, s0:s0 + nchk] = d2.reshape(nchk, P).T
            ea[:, s0:s0 + nchk, :] = a2.reshape(nchk, P, F_EDGE).transpose(1, 0, 2)

        naT = np.zeros((F_NODE, PN), np.float32)
        naT[:, :NPC] = node_attr[c * NPC:(c + 1) * NPC].T
        gloc = np.zeros((P, TILES), np.float32)
        gl = (batch[c * NPC:(c + 1) * NPC] - g_lo[c]).astype(np.float32)
        gl_pad = np.zeros(PN, np.float32)
        gl_pad[:NPC] = gl
        gloc[:, :] = gl_pad.reshape(TILES, P).T

        pe_, qe_ = np.meshgrid(np.arange(P), np.arange(NECH), indexing="ij")
        Me = np.zeros((P, NECH, P), ml_dtypes.bfloat16)
        Me[pe_, qe_, dloc] = np.asarray(1.0, ml_dtypes.bfloat16)
        im = dict(naT=naT, ea=ea.astype(BF), Mg=Mg, Me=Me, gidx=gidx, gloc=gloc)
        im.update(weights)
        in_maps.append(im)

    static = dict(NCH=NCH, NECH=NECH, CNT=CNT, ECNT=ECNT, e_off=e_off,
                  calls=calls, chunk_tile=chunk_tile,
                  first_chunk=first_chunk, last_chunk=last_chunk, g_lo=g_lo)
    return static, in_maps


def _build(st):
    NCH, NECH = st["NCH"], st["NECH"]
    CNT, ECNT, e_off = st["CNT"], st["ECNT"], st["e_off"]
    calls, chunk_tile = st["calls"], st["chunk_tile"]
    first_chunk, last_chunk = st["first_chunk"], st["last_chunk"]
    g_lo = st["g_lo"]
    f32 = mybir.dt.float32
    bf16 = mybir.dt.bfloat16

    nc = bacc.Bacc(None, num_devices=NC, debug=False, num_swdge_queues=4)
    # ---- inputs ----
    naT_d = nc.dram_tensor("naT", [F_NODE, PN], f32, kind="ExternalInput")
    ea_d = nc.dram_tensor("ea", [P, NECH, F_EDGE], bf16, kind="ExternalInput")
    Mg_d = nc.dram_tensor("Mg", [P, NCH, P], bf16, kind="ExternalInput")
    Me_d = nc.dram_tensor("Me", [P, NECH, P], bf16, kind="ExternalInput")
    gidx_d = nc.dram_tensor("gidx", [P, NCH * 8], mybir.dt.int16, kind="ExternalInput")
    gloc_d = nc.dram_tensor("gloc", [P, TILES], f32, kind="ExternalInput")
    Wn_d = nc.dram_tensor("W_node", [F_NODE, HID], f32, kind="ExternalInput")
    We_d = nc.dram_tensor("W_edge", [F_EDGE, HID], f32, kind="ExternalInput")
    bemb_d = nc.dram_tensor("b_embed", [1, HID], f32, kind="ExternalInput")
    Wl_d = [nc.dram_tensor(f"W{i}", [HID, HID], f32, kind="ExternalInput") for i in (1, 2, 3)]
    Wlb_d = [nc.dram_tensor(f"W{i}b", [HID, HID], bf16, kind="ExternalInput") for i in (1, 2, 3)]
    bl_d = [nc.dram_tensor(f"b{i}", [1, HID], f32, kind="ExternalInput") for i in (1, 2, 3)]
    Wp1_d = nc.dram_tensor("Wp1", [HID, HID], f32, kind="ExternalInput")
    bp1_d = nc.dram_tensor("bp1", [1, HID], f32, kind="ExternalInput")
    Wp2_d = nc.dram_tensor("Wp2", [HID, 1], f32, kind="ExternalInput")
    bp2_d = nc.dram_tensor("bp2", [1, 1], f32, kind="ExternalInput")
    out_d = nc.dram_tensor("out", [N_GRAPHS, 1], f32, kind="ExternalOutput")

    MAXCALL = max(nn for gc in calls for (_w, _s, nn) in gc)

    with TileContext(nc) as tc:
        with tc.tile_pool(name="persist", bufs=1) as pp, \
             tc.tile_pool(name="dram", bufs=1, space="DRAM") as dr:
            hT = pp.tile([P, PN], bf16)                     # resident h (feat-major)
            gloc_sb = pp.tile([P, TILES], f32)
            iota_sb = pp.tile([P, P], bf16)
            iota512_sb = pp.tile([P, G_WIN], f32)
            ones_sb = pp.tile([1, G_WIN], f32)
            ident_sb = pp.tile([P, P], bf16)
            Wn_sb = pp.tile([F_NODE, HID], f32)
            We_sb = pp.tile([F_EDGE, HID], f32)
            bemb_sb = pp.tile([1, HID], f32)
            Wl_sb = [pp.tile([HID, HID], bf16, name=f"Wl{i}") for i in range(3)]
            bl_sb = [pp.tile([1, HID], f32, name=f"bl{i}") for i in range(3)]
            Wp1_sb = pp.tile([HID, HID], f32)
            bp1_sb = pp.tile([1, HID], f32)
            Wp2_sb = pp.tile([HID, 1], f32)
            bp2_sb = pp.tile([1, 1], f32)

            hwslice = dr.tile([PN, HID], bf16)
            tables = [dr.tile([TBL, HID], bf16, addr_space="Shared", name=f"table{l}") for l in range(3)]
            fpslice = dr.tile([P, G_WIN], f32)
            fp_all = dr.tile([NC * P, G_WIN], f32, addr_space="Shared")

            nc.sync.dma_start(gloc_sb[:], gloc_d[:])
            nc.sync.dma_start(Wn_sb[:], Wn_d[:])
            nc.sync.dma_start(We_sb[:], We_d[:])
            nc.sync.dma_start(bemb_sb[:], bemb_d[:])
            for i in range(3):
                nc.sync.dma_start(Wl_sb[i][:], Wlb_d[i][:])
                nc.sync.dma_start(bl_sb[i][:], bl_d[i][:])
            nc.sync.dma_start(Wp1_sb[:], Wp1_d[:])
            nc.sync.dma_start(bp1_sb[:], bp1_d[:])
            nc.sync.dma_start(Wp2_sb[:], Wp2_d[:])
            nc.sync.dma_start(bp2_sb[:], bp2_d[:])
            nc.gpsimd.iota(iota_sb[:], pattern=[[1, P]], base=0,
                           channel_multiplier=0, allow_small_or_imprecise_dtypes=True)
            nc.gpsimd.iota(iota512_sb[:], pattern=[[1, G_WIN]], base=0,
                           channel_multiplier=0, allow_small_or_imprecise_dtypes=True)
            nc.gpsimd.memset(ones_sb[:], 1.0)
            make_identity(nc, ident_sb[:])

            # Shared pools for phase A (hw = h @ W + b), open across all stages
            with tc.tile_pool(name="g_hw", bufs=3) as ghw, \
                 tc.tile_pool(name="g_call", bufs=6) as gcp, \
                 tc.tile_pool(name="g_m", bufs=4) as gmp, \
                 tc.tile_pool(name="p_sb", bufs=4) as psb, \
                 tc.tile_pool(name="g_psa", bufs=1, space="PSUM") as psa:

                def phase_a(layer, t):
                    # hw(layer) for tile t -> hwslice rows (bf16), node-major
                    apsum = psa.tile([P, HID], f32, tag="hw", name=f"hw{layer}_{t}")
                    nc.tensor.matmul(apsum[:], hT[:, t * P:(t + 1) * P],
                                     Wl_sb[layer][:], start=True, stop=False)
                    nc.tensor.matmul(apsum[:], ones_sb[:, :P], bl_sb[layer][:],
                                     start=False, stop=True)
                    hw_sb = ghw.tile([P, HID], bf16, tag="hwsb", name=f"hwsb{layer}_{t}")
                    nc.vector.tensor_copy(hw_sb[:], apsum[:])
                    nc.sync.dma_start(hwslice[t * P:(t + 1) * P, :], hw_sb[:])

                def emit_ag(layer):
                    nc.gpsimd.collective_compute(
                        "AllGather", mybir.AluOpType.bypass,
                        replica_groups=[list(range(NC))],
                        ins=[hwslice[:].opt()], outs=[tables[layer][:].opt()])

                # ================= Stage 1: attr embedder (+ phase A of layer 0) ====
                with tc.tile_pool(name="e_sb", bufs=2) as esb, \
                     tc.tile_pool(name="e_na", bufs=3) as ena, \
                     tc.tile_pool(name="e_m", bufs=2) as emp, \
                     tc.tile_pool(name="e_q", bufs=3) as eqp, \
                     tc.tile_pool(name="e_psq", bufs=2, space="PSUM") as psq, \
                     tc.tile_pool(name="e_pso", bufs=4, space="PSUM") as pso:
                    for g in range(NEGRP):
                        t0, t1 = g * EGRP, min((g + 1) * EGRP, TILES)
                        c0, c1 = int(e_off[t0]), int(e_off[t1])
                        if c1 > c0:
                            ea_sb = esb.tile([P, c1 - c0, F_EDGE], bf16, tag="ea")
                            nc.sync.dma_start(ea_sb[:], ea_d[:, c0:c1, :])
                        for t in range(t0, t1):
                            nchk = int(ECNT[t])
                            qpsum = psq.tile([F_EDGE, P], f32, tag="q")
                            if nchk:
                                e0 = int(e_off[t])
                                m01 = emp.tile([P, nchk, P], bf16, tag="m01")
                                nc.sync.dma_start(m01[:], Me_d[:, e0:e0 + nchk, :])
                            for j in range(nchk):
                                q = int(e_off[t]) + j
                                nc.tensor.matmul(
                                    qpsum[:], ea_sb[:, q - c0, :], m01[:, j, :],
                                    start=(j == 0), stop=(j == nchk - 1))
                            q_sb = eqp.tile([F_EDGE, P], f32, tag="qsb")
                            if nchk:
                                nc.vector.tensor_copy(q_sb[:], qpsum[:])
                            else:
                                nc.gpsimd.memset(q_sb[:], 0.0)
                            na_sb = ena.tile([F_NODE, P], f32, tag="na")
                            nc.sync.dma_start(na_sb[:], naT_d[:, t * P:(t + 1) * P])
                            opsum = pso.tile([P, P], f32, tag="eo")
                            nc.tensor.matmul(opsum[:], We_sb[:], q_sb[:], start=True, stop=False)
                            nc.tensor.matmul(opsum[:], Wn_sb[:], na_sb[:], start=False, stop=False)
                            nc.tensor.matmul(opsum[:], bemb_sb[:], ones_sb[:, :P], start=False, stop=True)
                            nc.scalar.activation(hT[:, t * P:(t + 1) * P], opsum[:],
                                                 mybir.ActivationFunctionType.Relu)
                            phase_a(0, t)
                emit_ag(0)

                # ================= Stage 2: GCN layers (phase A/pooling interleaved) =
                with tc.tile_pool(name="g_psc", bufs=4, space="PSUM") as psc, \
                     tc.tile_pool(name="p_ps", bufs=2, space="PSUM") as pst, \
                     tc.tile_pool(name="p_fp", bufs=1, space="PSUM") as pfp:
                    fp_psum = pfp.tile([P, G_WIN], f32)

                    def pool_tile(t):
                        tpsum = pst.tile([P, P], bf16, tag="tp", name=f"tp{t}")
                        nc.tensor.transpose(out=tpsum[:], in_=hT[:, t * P:(t + 1) * P],
                                            identity=ident_sb[:])
                        hnm = psb.tile([P, P], f32, tag="hnm", bufs=3, name=f"hnm{t}")
                        nc.vector.tensor_copy(hnm[:], tpsum[:])
                        b01 = psb.tile([P, G_WIN], f32, tag="b01", bufs=3, name=f"b01{t}")
                        nc.vector.tensor_scalar(
                            out=b01[:], in0=iota512_sb[:],
                            scalar1=gloc_sb[:, t:t + 1], scalar2=None,
                            op0=mybir.AluOpType.is_equal)
                        nc.tensor.matmul(fp_psum[:], hnm[:], b01[:],
                                         start=(t == 0), stop=(t == TILES - 1),
                                         skip_group_check=True)

                    for layer in range(3):
                        for g in range(NGRP):
                            t0, t1 = g * GRP, min((g + 1) * GRP, TILES)
                            g0 = calls[g][0][1]
                            gn = calls[g][-1][1] + calls[g][-1][2] - g0
                            gix = gmp.tile([P, gn * 8], mybir.dt.int16, tag="gix")
                            nc.sync.dma_start(gix[:], gidx_d[:, g0 * 8:(g0 + gn) * 8])
                            cpsum = {t: psc.tile([P, P], f32, tag="cp", name=f"cp{layer}_{t}")
                                     for t in range(t0, t1)}
                            for (w, s, nn) in calls[g]:
                                gbuf = gcp.tile([P, MAXCALL, P], bf16, tag="gc")
                                base = w * WIN
                                rows = min(WIN, TBL - base)
                                nc.gpsimd.dma_gather(
                                    out_ap=gbuf[:, :nn, :],
                                    in_ap=tables[layer][base:base + rows, :],
                                    idxs_ap=gix[:, (s - g0) * 8:(s - g0 + nn) * 8],
                                    num_idxs=nn * P, num_idxs_reg=nn * P,
                                    elem_size=HID, single_packet=False, queue_num=w)
                                mt = gmp.tile([P, MAXCALL, P], bf16, tag="m")
                                nc.sync.dma_start(mt[:, :nn, :], Mg_d[:, s:s + nn, :])
                                for j in range(nn):
                                    q = s + j
                                    t = chunk_tile[q]
                                    nc.tensor.matmul(
                                        cpsum[t][:], gbuf[:, j, :], mt[:, j, :],
                                        start=(q == first_chunk[t]),
                                        stop=(q == last_chunk[t]))
                            for t in range(t0, t1):
                                nc.scalar.activation(hT[:, t * P:(t + 1) * P],
                                                     cpsum[t][:],
                                                     mybir.ActivationFunctionType.Relu)
                                if layer < 2:
                                    phase_a(layer + 1, t)
                                else:
                                    pool_tile(t)
                        if layer < 2:
                            emit_ag(layer + 1)

                # ================= Stage 3: pooled-sum exchange + MLP ================
                with tc.tile_pool(name="p_mlp", bufs=2, space="PSUM") as pml:
                    fp_sb = psb.tile([P, G_WIN], f32, tag="fpsb", bufs=1)
                    nc.vector.tensor_copy(fp_sb[:], fp_psum[:])
                    nc.sync.dma_start(fpslice[:], fp_sb[:])
                    nc.gpsimd.collective_compute(
                        "AllGather", mybir.AluOpType.bypass,
                        replica_groups=[list(range(NC))],
                        ins=[fpslice[:].opt()], outs=[fp_all[:].opt()])
                    # static assembly of full fpT [128, N_GRAPHS]
                    fpT = psb.tile([P, N_GRAPHS], f32, tag="fpT", bufs=1)
                    nc.vector.memset(fpT[:], 0.0)
                    for c in range(NC):
                        wdt = min(G_WIN, N_GRAPHS - g_lo[c])
                        part = psb.tile([P, G_WIN], f32, tag="fpp", bufs=2)
                        nc.sync.dma_start(part[:], fp_all[c * P:(c + 1) * P, :])
                        nc.vector.tensor_add(fpT[:, g_lo[c]:g_lo[c] + wdt],
                                             fpT[:, g_lo[c]:g_lo[c] + wdt],
                                             part[:, :wdt])
                    # z = relu(fp @ Wp1 + bp1); out = z @ Wp2 + bp2
                    zT = psb.tile([P, N_GRAPHS], f32, tag="zT", bufs=1)
                    o_sb = psb.tile([1, N_GRAPHS], f32, tag="osb", bufs=1)
                    for j in range(N_GRAPHS // G_WIN):
                        sl = slice(j * G_WIN, (j + 1) * G_WIN)
                        zp = pml.tile([P, G_WIN], f32, tag="zp")
                        nc.tensor.matmul(zp[:], Wp1_sb[:], fpT[:, sl], start=True, stop=False)
                        nc.tensor.matmul(zp[:], bp1_sb[:], ones_sb[:], start=False, stop=True)
                        nc.scalar.activation(zT[:, sl], zp[:], mybir.ActivationFunctionType.Relu)
                        op = pml.tile([1, G_WIN], f32, tag="op")
                        nc.tensor.matmul(op[:], Wp2_sb[:], zT[:, sl], start=True, stop=False)
                        nc.tensor.matmul(op[:], bp2_sb[:], ones_sb[:], start=False, stop=True)
                        nc.vector.tensor_copy(o_sb[:, sl], op[:])
                    nc.sync.dma_start(out_d[:, 0][None, :], o_sb[:])
    nc.finalize()
    return nc


def kernel(**inputs):
    global LAST_RESULT
    _install_ntff_hook()
    st, in_maps = _prep(inputs)
    key = (st["NCH"], st["NECH"])
    if key not in _CACHE:
        _CACHE[key] = _build(st)
    nc = _CACHE[key]
    import os
    trace = bool(os.environ.get("BASS_TRACE"))
    r = run_bass_kernel_spmd(nc, in_maps, core_ids=list(range(NC)), trace=trace)
    LAST_RESULT = r
    return r.results[0]["out"]
